# revision 3
# baseline (speedup 1.0000x reference)
"""Tacotron2-style decoder (nn_Decoder_49289044688964) as a Trainium2 Bass kernel.

Strategy (8 NeuronCores, data-parallel over batch B=256 -> 32/core, weights
replicated, no collectives):

  Phase A (parallel over time): Z = X @ W_ih.T + (b_ih + b_hh) for all 50
    decoder steps at once, fp32r matmuls (full PE speed at N>=256).
  Phase B (serial, 50 steps): gates = Z_t + h @ W_hh.T in bf16 with 4-way
    tile_position col-group packing (M=32 batch per group), LSTM pointwise in
    an "S4" stacked layout ([128,256] = 4 column-quarters stacked on
    partitions, one PSUM tile per gate pair -> all elementwise ops stay
    partition-aligned), Z_t added in fp32 via DVE, h transposed back to hT
    via 4-way-concurrent PE transposes (one PSUM bank per row group).
  Phase C (parallel): [mel | gate] = H @ [W_projH | W_gateH].T + ctx terms,
    fp32r, with the context projection computed once on device.

Numerics: fp32r = fp32 with 11 explicit mantissa bits (verified on HW);
bf16 only on the h @ W_hh recurrent term; everything accumulates in fp32.
"""
import numpy as np
import ml_dtypes

import concourse.bass as bass
import concourse.mybir as mybir
import concourse.tile as tile
from concourse import bacc
from concourse.bass_utils import run_bass_kernel_spmd

f32 = mybir.dt.float32
f32r = mybir.dt.float32r
bf16 = mybir.dt.bfloat16
AF = mybir.ActivationFunctionType

B, N_MEL, T_MEL = 256, 80, 800
R = 16
T = T_MEL // R            # 50 decoder steps
RNN = 1024
ENC = 256
IN_DIM = RNN + ENC        # 1280
MEL = N_MEL * R           # 1280
G = 4 * RNN               # 4096
NCORES = 8
BL = B // NCORES          # 32 batch rows per core
ROWS = T * BL             # 1600 (t, b) rows per core
MELX = MEL + 2            # 1282: mel + 1 gate col + 1 zero pad (fp32r needs even N)
RCHUNKS = [(i * 128, min(128, ROWS - i * 128)) for i in range((ROWS + 127) // 128)]
NCHUNKS_C = [(0, 512), (512, 512), (1024, 258)]   # MELX column chunks


def _round_f32r(x: np.ndarray) -> np.ndarray:
    """Round-to-nearest-even to fp32r (11 explicit mantissa bits)."""
    u = np.ascontiguousarray(x, dtype=np.float32).view(np.uint32)
    r = (u + 0x7FF + ((u >> 12) & 1)) & np.uint32(0xFFFFF000)
    return r.view(np.float32)


def build_nc():
    nc = bacc.Bacc("TRN2", target_bir_lowering=False, debug=False,
                   num_devices=NCORES)

    xT = nc.dram_tensor("xT", [128, 10, ROWS], f32r, kind="ExternalInput")
    wih = nc.dram_tensor("wih", [128, 10, G], f32r, kind="ExternalInput")
    bsum = nc.dram_tensor("bsum", [128, G], f32, kind="ExternalInput")
    whh = nc.dram_tensor("whh", [128, 8, 4, 1024], bf16, kind="ExternalInput")
    wpe = nc.dram_tensor("wpe", [128, 8, MELX], f32r, kind="ExternalInput")
    ctxT4 = nc.dram_tensor("ctxT4", [128, 2, 128], f32r, kind="ExternalInput")
    wce = nc.dram_tensor("wce", [128, 2, MELX], f32r, kind="ExternalInput")
    bias_pc = nc.dram_tensor("bias_pc", [128, MELX], f32, kind="ExternalInput")
    id_f = nc.dram_tensor("id_f", [128, 32], f32, kind="ExternalInput")
    mel_ext = nc.dram_tensor("mel_ext", [ROWS, MELX], f32, kind="ExternalOutput")

    with tile.TileContext(nc) as tc:
        with tc.tile_pool(name="dram", bufs=1, space="DRAM") as dpool, \
             tc.tile_pool(name="keep", bufs=1) as keep:
            zd = dpool.tile([T, BL, G], f32r)       # Z = X@W_ih.T + bias
            ht_d = dpool.tile([8, 128, T, BL], f32r)  # H.T by rnn k-chunk

            whh_sb = keep.tile([128, 8, 4, 1024], bf16)
            nc.sync.dma_start(whh_sb[:], whh[:])
            idf = keep.tile([128, 32], f32)
            nc.sync.dma_start(idf[:], id_f[:])

            # ---------------- Phase A: Z = X @ W_ih.T + bsum ----------------
            with tc.tile_pool(name="a_xt", bufs=1) as p_xt, \
                 tc.tile_pool(name="a_w", bufs=2) as p_w, \
                 tc.tile_pool(name="a_bs", bufs=2) as p_bs, \
                 tc.tile_pool(name="a_o", bufs=3) as p_o, \
                 tc.tile_pool(name="a_ps", bufs=2, space="PSUM") as p_ps:
                xt_sb = p_xt.tile([128, 10, ROWS], f32r)
                nc.sync.dma_start(xt_sb[:], xT[:])
                for n in range(8):
                    w_t = p_w.tile([128, 10, 512], f32r, tag="a_w")
                    nc.sync.dma_start(w_t[:], wih[:, :, n * 512:(n + 1) * 512])
                    bs_t = p_bs.tile([128, 512], f32, tag="a_bs")
                    nc.sync.dma_start(bs_t[:], bsum[:, n * 512:(n + 1) * 512])
                    for r0, m in RCHUNKS:
                        ps = p_ps.tile([128, 512], f32, tag="a_ps")
                        for k in range(10):
                            nc.tensor.matmul(ps[:m, :], xt_sb[:, k, r0:r0 + m],
                                             w_t[:, k, :],
                                             start=(k == 0), stop=(k == 9))
                        z_t = p_o.tile([128, 512], f32r, tag="a_z")
                        nc.vector.tensor_add(z_t[:m, :], ps[:m, :], bs_t[:m, :])
                        nc.sync.dma_start(
                            zd[:].rearrange("t b g -> (t b) g")
                              [r0:r0 + m, n * 512:(n + 1) * 512],
                            z_t[:m, :])

            # ---------------- Phase B: 50 serial LSTM steps ----------------
            with tc.tile_pool(name="b_st", bufs=1) as b_st, \
                 tc.tile_pool(name="b_z", bufs=3) as b_z, \
                 tc.tile_pool(name="b_pw", bufs=2) as b_pw, \
                 tc.tile_pool(name="b_ht", bufs=2) as b_ht, \
                 tc.tile_pool(name="b_ps", bufs=2, space="PSUM") as b_ps, \
                 tc.tile_pool(name="b_pt", bufs=1, space="PSUM") as b_pt:
                c_sb = b_st.tile([128, 256], f32)
                nc.vector.memset(c_sb[:], 0.0)
                hT_state = [b_st.tile([128, 256], bf16, name=f"hT_s{i}", tag=f"hT_s{i}")
                            for i in range(2)]
                nc.vector.memset(hT_state[0][:], 0.0)

                for t in range(T):
                    hT_cur = hT_state[t % 2]
                    hT_nxt = hT_state[(t + 1) % 2]

                    z_if = b_z.tile([128, 512], f32, tag="z_if")
                    z_go = b_z.tile([128, 512], f32, tag="z_go")
                    for q in range(4):
                        for ztile, base in ((z_if, 0), (z_go, 2048)):
                            nc.sync.dma_start(
                                ztile[q * 32:(q + 1) * 32, :]
                                .rearrange("p (h j) -> p h j", h=2),
                                zd[t, :, base:base + 2048].bitcast(f32)
                                .rearrange("b (h x) -> b h x", h=2)
                                [:, :, q * 256:(q + 1) * 256])

                    P1 = b_ps.tile([128, 512], f32, tag="P1")  # [i|f] quarters
                    P2 = b_ps.tile([128, 512], f32, tag="P2")  # [g|o] quarters
                    for k in range(8):
                        for pi, P in enumerate((P1, P2)):
                            for q in range(4):
                                nc.tensor.matmul(
                                    P[q * 32:(q + 1) * 32, :],
                                    hT_cur[:, k * 32:(k + 1) * 32],
                                    whh_sb[:, k, q, pi * 512:(pi + 1) * 512],
                                    start=(k == 0), stop=(k == 7),
                                    tile_position=(0, q * 32))

                    s_if = b_pw.tile([128, 512], f32, tag="s_if")
                    nc.vector.tensor_add(s_if[:], P1[:], z_if[:])
                    s_go = b_pw.tile([128, 512], f32, tag="s_go")
                    nc.vector.tensor_add(s_go[:], P2[:], z_go[:])

                    sig1 = b_pw.tile([128, 512], f32, tag="sig1")
                    nc.scalar.activation(sig1[:], s_if[:], AF.Sigmoid)
                    tg = b_pw.tile([128, 256], f32, tag="tg")
                    nc.scalar.activation(tg[:], s_go[:, 0:256], AF.Tanh)
                    so = b_pw.tile([128, 256], f32, tag="so")
                    nc.scalar.activation(so[:], s_go[:, 256:512], AF.Sigmoid)

                    t2 = b_pw.tile([128, 256], f32, tag="t2")
                    nc.vector.tensor_mul(t2[:], sig1[:, 0:256], tg[:])
                    t1 = b_pw.tile([128, 256], f32, tag="t1")
                    nc.vector.tensor_mul(t1[:], sig1[:, 256:512], c_sb[:])
                    nc.vector.tensor_add(c_sb[:], t1[:], t2[:])
                    tc_t = b_pw.tile([128, 256], f32, tag="tc_t")
                    nc.scalar.activation(tc_t[:], c_sb[:], AF.Tanh)
                    h_sb = b_pw.tile([128, 256], f32, tag="h_sb")
                    nc.vector.tensor_mul(h_sb[:], so[:], tc_t[:])

                    Pt = [b_pt.tile([128, 64], f32, tag=f"Pt{q}", name=f"Pt{q}")
                          for q in range(4)]
                    for c8 in range(8):
                        q, colh = c8 // 2, (c8 % 2) * 128
                        nc.tensor.transpose(
                            Pt[q][:, (c8 % 2) * 32:(c8 % 2) * 32 + 32],
                            h_sb[q * 32:(q + 1) * 32, colh:colh + 128],
                            idf[q * 32:(q + 1) * 32, :],
                            tile_position=(q * 32, 0))
                    hT_pc = b_ht.tile([128, 256], f32r, tag="hT_pc")
                    for q in range(4):
                        nc.vector.tensor_copy(hT_nxt[:, q * 64:(q + 1) * 64],
                                              Pt[q][:])
                        nc.vector.tensor_copy(hT_pc[:, q * 64:(q + 1) * 64],
                                              Pt[q][:])
                    nc.sync.dma_start(
                        ht_d[:, :, t, :].rearrange("k p b -> p k b"),
                        hT_pc[:].rearrange("p (k b) -> p k b", k=8))

            # ---------------- Phase C: [mel|gate] = H @ Wext.T + ctx --------
            with tc.tile_pool(name="c_w", bufs=1) as c_w, \
                 tc.tile_pool(name="c_h", bufs=16) as c_h, \
                 tc.tile_pool(name="c_o", bufs=3) as c_o, \
                 tc.tile_pool(name="c_ps", bufs=2, space="PSUM") as c_ps:
                # context projection, once: ctx4 [128, MELX]
                ctx_sb = c_w.tile([128, 2, 128], f32r)
                nc.sync.dma_start(ctx_sb[:], ctxT4[:])
                wce_sb = c_w.tile([128, 2, MELX], f32r)
                nc.sync.dma_start(wce_sb[:], wce[:])
                bias_sb = c_w.tile([128, MELX], f32)
                nc.sync.dma_start(bias_sb[:], bias_pc[:])
                ctx4 = c_w.tile([128, MELX], f32)
                for n0, nsz in NCHUNKS_C:
                    psx = c_ps.tile([128, 512], f32, tag="c_ps")
                    for k in range(2):
                        nc.tensor.matmul(psx[:, :nsz], ctx_sb[:, k, :],
                                         wce_sb[:, k, n0:n0 + nsz],
                                         start=(k == 0), stop=(k == 1))
                    nc.vector.tensor_add(ctx4[:, n0:n0 + nsz], psx[:, :nsz],
                                         bias_sb[:, n0:n0 + nsz])

                wpe_sb = c_w.tile([128, 8, MELX], f32r)
                nc.sync.dma_start(wpe_sb[:], wpe[:])
                for r0, m in RCHUNKS:
                    hts = []
                    for k in range(8):
                        ht_t = c_h.tile([128, 128], f32r, tag="c_ht")
                        nc.sync.dma_start(
                            ht_t[:, :m],
                            ht_d[k].rearrange("p t b -> p (t b)")[:, r0:r0 + m])
                        hts.append(ht_t)
                    for n0, nsz in NCHUNKS_C:
                        psx = c_ps.tile([128, 512], f32, tag="c_ps")
                        for k in range(8):
                            nc.tensor.matmul(psx[:m, :nsz], hts[k][:, :m],
                                             wpe_sb[:, k, n0:n0 + nsz],
                                             start=(k == 0), stop=(k == 7))
                        o_t = c_o.tile([128, 512], f32, tag="c_o")
                        nc.vector.tensor_add(o_t[:m, :nsz], psx[:m, :nsz],
                                             ctx4[:m, n0:n0 + nsz])
                        nc.sync.dma_start(mel_ext[r0:r0 + m, n0:n0 + nsz],
                                          o_t[:m, :nsz])
    nc.compile()
    return nc


_NC_CACHE = None


def _get_nc():
    global _NC_CACHE
    if _NC_CACHE is None:
        _NC_CACHE = build_nc()
    return _NC_CACHE


def _prep_inputs(context, target, W_ih, b_ih, W_hh, b_hh,
                 W_proj, b_proj, W_gate, b_gate):
    """Host-side layout prep -> list of 8 per-core input maps."""
    context = np.asarray(context, dtype=np.float32)
    target = np.asarray(target, dtype=np.float32)
    W_ih = np.asarray(W_ih, dtype=np.float32)
    b_ih = np.asarray(b_ih, dtype=np.float32)
    W_hh = np.asarray(W_hh, dtype=np.float32)
    b_hh = np.asarray(b_hh, dtype=np.float32)
    W_proj = np.asarray(W_proj, dtype=np.float32)
    b_proj = np.asarray(b_proj, dtype=np.float32)
    W_gate = np.asarray(W_gate, dtype=np.float32)
    b_gate = np.asarray(b_gate, dtype=np.float32)

    # teacher-forced inputs X [T, B, IN_DIM]; X[0] = 0 (go frame)
    frames = target.transpose(0, 2, 1).reshape(B, T, MEL)
    X = np.ascontiguousarray(frames.transpose(1, 0, 2))
    X[0] = 0.0

    # --- replicated weight tensors ---
    wih_h = _round_f32r(np.ascontiguousarray(
        W_ih.T.reshape(10, 128, G).transpose(1, 0, 2)))
    bsum_h = np.ascontiguousarray(
        np.broadcast_to(b_ih + b_hh, (128, G)))
    WhhT = np.ascontiguousarray(W_hh.T)                      # [RNN, G]
    whh_h = np.ascontiguousarray(
        WhhT.reshape(8, 128, 4, 4, 256).transpose(1, 0, 3, 2, 4)
        .reshape(128, 8, 4, 1024).astype(ml_dtypes.bfloat16))
    WextT = np.concatenate([W_proj[:, :RNN], W_gate[:, :RNN],
                            np.zeros((1, RNN), np.float32)], axis=0).T
    wpe_h = _round_f32r(np.ascontiguousarray(
        WextT.reshape(8, 128, MELX).transpose(1, 0, 2)))
    WceT = np.concatenate([W_proj[:, RNN:], W_gate[:, RNN:],
                           np.zeros((1, ENC), np.float32)], axis=0).T
    wce_h = _round_f32r(np.ascontiguousarray(
        WceT.reshape(2, 128, MELX).transpose(1, 0, 2)))
    bias_h = np.ascontiguousarray(
        np.broadcast_to(np.concatenate([b_proj, b_gate, np.zeros(1, np.float32)]),
                        (128, MELX)))
    idf_h = np.zeros((128, 32), np.float32)
    for q in range(4):
        idf_h[q * 32:(q + 1) * 32] = np.eye(32, dtype=np.float32)

    in_maps = []
    for c in range(NCORES):
        bsl = slice(c * BL, (c + 1) * BL)
        Xc = X[:, bsl, :]                                    # [T, BL, IN_DIM]
        xT_h = _round_f32r(np.ascontiguousarray(
            Xc.transpose(2, 0, 1).reshape(IN_DIM, ROWS)
            .reshape(10, 128, ROWS).transpose(1, 0, 2)))
        ctxT = context[bsl].T                                # [ENC, BL]
        ctxT4_h = _round_f32r(np.ascontiguousarray(
            np.tile(ctxT.reshape(2, 128, BL), (1, 1, 4)).transpose(1, 0, 2)))
        in_maps.append({
            "xT": xT_h, "wih": wih_h, "bsum": bsum_h, "whh": whh_h,
            "wpe": wpe_h, "ctxT4": ctxT4_h, "wce": wce_h,
            "bias_pc": bias_h, "id_f": idf_h,
        })
    return in_maps


def _assemble(results):
    """8 per-core mel_ext [ROWS, MELX] -> (mel_outputs, gate_outputs)."""
    mel_full = np.empty((T, B, MEL), dtype=np.float32)
    gate_full = np.empty((T, B, 1), dtype=np.float32)
    for c, res in enumerate(results):
        me = res["mel_ext"].reshape(T, BL, MELX)
        bsl = slice(c * BL, (c + 1) * BL)
        mel_full[:, bsl, :] = me[:, :, :MEL]
        gate_full[:, bsl, 0] = me[:, :, MEL]
    mel_outputs = np.ascontiguousarray(
        mel_full.transpose(1, 0, 2).reshape(B, T * R, N_MEL).transpose(0, 2, 1))
    gate_outputs = np.ascontiguousarray(gate_full.transpose(1, 0, 2))
    return mel_outputs, gate_outputs


def kernel(**inputs):
    in_maps = _prep_inputs(**inputs)
    nc = _get_nc()
    out = run_bass_kernel_spmd(nc, in_maps, list(range(NCORES)))
    return _assemble(out.results)


if __name__ == "__main__":
    rng = np.random.default_rng(0)
    s = 0.02
    inputs = {
        "context": rng.standard_normal((B, ENC)).astype(np.float32),
        "target": rng.standard_normal((B, N_MEL, T_MEL)).astype(np.float32),
        "W_ih": (rng.standard_normal((G, IN_DIM)) * s).astype(np.float32),
        "b_ih": np.zeros((G,), np.float32),
        "W_hh": (rng.standard_normal((G, RNN)) * s).astype(np.float32),
        "b_hh": np.zeros((G,), np.float32),
        "W_proj": (rng.standard_normal((MEL, IN_DIM)) * s).astype(np.float32),
        "b_proj": np.zeros((MEL,), np.float32),
        "W_gate": (rng.standard_normal((1, IN_DIM)) * s).astype(np.float32),
        "b_gate": np.zeros((1,), np.float32),
    }
    mel, gate = kernel(**inputs)
    print("mel:", mel.shape, mel.dtype, "gate:", gate.shape, gate.dtype)
